# revision 15
# baseline (speedup 1.0000x reference)
"""Multi-head attention (B=2, S=4096, D=512, H=8) on 8 NeuronCores.

Sharding: data-parallel on batch x head-pair-parallel.  Core c handles
batch b = c//4 and heads (2*(c%4), 2*(c%4)+1).  Each core computes its
[4096, 128] slice of the output; the host scatters inputs / gathers
outputs.

Per-core kernel (Bass/Tile), operands in fp16 (fp32 PSUM accumulate):
  - Pipelined prologue: x^T chunks DMA in while K^T and V projections
    run per 512-wide chunk; then Q^T projections.
  - Q^T/K^T are [128(d of 2 heads), S] with head dims on partitions; V
    sits in natural [k, d] layout padded to 128-wide FWL weight tiles
    with a ones column (so the E@V matmul also emits softmax row sums).
  - Attention per 512-wide q chunk / 128-wide k tile:
      S^T tile  = K^T.T @ Q^T  (two row-packed K=64 matmuls)
      E         = exp(S^T / 8) (ACT instrs over [128, 1536] PSUM chunks)
      O^T      += V1.T @ E     (PSUM accumulate, row 64 = row sums)
  - Output: PE transpose of O^T, reciprocal of row sums, per-partition
    scale, DMA out.
"""

import numpy as np

N_CORES = 8
S_FULL = 4096
D_MODEL = 512
HEAD = 64

_cached = {}


def build_nc(S=S_FULL):
    import concourse.bass as bass
    from concourse import bacc
    import concourse.mybir as mybir
    import concourse.tile as tile
    f32 = mybir.dt.float32
    f16 = mybir.dt.float16
    AF = mybir.ActivationFunctionType

    D = D_MODEL
    n_qc = S // 512     # 512-wide query chunks
    n_kc = S // 128     # 128-wide key tiles
    n_dc = D // 128     # 128-wide contraction chunks of D

    nc = bacc.Bacc()

    xT = nc.dram_tensor("xT", [D, S], f16, kind="ExternalInput")
    wqT = nc.dram_tensor("wqT", [D, 128], f16, kind="ExternalInput")
    wkT = nc.dram_tensor("wkT", [D, 128], f16, kind="ExternalInput")
    wvT = nc.dram_tensor("wvT", [D, 130], f16, kind="ExternalInput")
    bq = nc.dram_tensor("bq", [128, 1], f32, kind="ExternalInput")
    bk = nc.dram_tensor("bk", [128, 1], f32, kind="ExternalInput")
    bvb = nc.dram_tensor("bvb", [128, 130], f32, kind="ExternalInput")
    out = nc.dram_tensor("out", [S, 128], f32, kind="ExternalOutput")

    with tile.TileContext(nc) as tc:
        with (
            tc.tile_pool(name="consts", bufs=1) as consts,
            tc.tile_pool(name="persist", bufs=1) as persist,
        ):
            wq_sb = consts.tile([128, n_dc * 128], f16, name="wq_sb")
            wk_sb = consts.tile([128, n_dc * 128], f16, name="wk_sb")
            wv_sb = consts.tile([128, n_dc * 130], f16, name="wv_sb")
            bq_sb = consts.tile([128, 1], f32, name="bq_sb")
            bk_sb = consts.tile([128, 1], f32, name="bk_sb")
            bvb_sb = consts.tile([128, 130], f32, name="bvb_sb")
            for dc in range(n_dc):
                r = slice(dc * 128, (dc + 1) * 128)
                nc.sync.dma_start(wq_sb[:, dc * 128:(dc + 1) * 128], wqT[r, :])
                nc.sync.dma_start(wk_sb[:, dc * 128:(dc + 1) * 128], wkT[r, :])
                nc.sync.dma_start(wv_sb[:, dc * 130:(dc + 1) * 130], wvT[r, :])
            nc.sync.dma_start(bq_sb[:], bq[:, :])
            nc.sync.dma_start(bk_sb[:], bk[:, :])
            nc.sync.dma_start(bvb_sb[:], bvb[:, :])

            xt = [persist.tile([128, S], f16, name=f"xt{i}") for i in range(n_dc)]
            qt = persist.tile([128, S], f16, name="qt")
            kt = persist.tile([128, S], f16, name="kt")
            # V1[kc*256 + h*128 : +65] = [V_h | ones]; rest zero padding so
            # every E@V weight tile is a full 128-column (FWL-eligible) load.
            v1 = persist.tile([128, n_kc * 256], f16, name="v1")
            nc.vector.memset(v1[:], 0.0)

            # ---- prologue: x^T DMA + K/V projections pipelined per block --
            blk = 1024 if S >= 1024 else 512
            n_blk = S // blk
            with tc.tile_pool(name="pproj", bufs=2, space="PSUM") as pproj:
                for c in range(n_blk):
                    cs = slice(c * blk, (c + 1) * blk)
                    for dc in range(n_dc):
                        nc.sync.dma_start(
                            xt[dc][:, cs], xT[dc * 128:(dc + 1) * 128, cs]
                        )
                    for half in range(blk // 512):
                        hs = slice(c * blk + half * 512, c * blk + (half + 1) * 512)
                        pk = pproj.tile([128, 512], f32, name="pk", tag="pk")
                        for dc in range(n_dc):
                            nc.tensor.matmul(
                                pk[:],
                                lhsT=wk_sb[:, dc * 128:(dc + 1) * 128],
                                rhs=xt[dc][:, hs],
                                start=(dc == 0),
                                stop=(dc == n_dc - 1),
                            )
                        nc.vector.tensor_scalar_add(kt[:, hs], pk[:], bk_sb[:])
                    for sti in range(blk // 128):
                        st_ = c * (blk // 128) + sti
                        ss = slice(st_ * 128, (st_ + 1) * 128)
                        pv = pproj.tile([128, 130], f32, name="pv", tag="pv")
                        for dc in range(n_dc):
                            nc.tensor.matmul(
                                pv[:],
                                lhsT=xt[dc][:, ss],
                                rhs=wv_sb[:, dc * 130:(dc + 1) * 130],
                                start=(dc == 0),
                                stop=(dc == n_dc - 1),
                            )
                        for h in range(2):
                            nc.vector.tensor_add(
                                v1[:, st_ * 256 + h * 128: st_ * 256 + h * 128 + 65],
                                pv[:, h * 65:(h + 1) * 65],
                                bvb_sb[:, h * 65:(h + 1) * 65],
                            )
                for c in range(n_qc):
                    cs = slice(c * 512, (c + 1) * 512)
                    pq = pproj.tile([128, 512], f32, name="pq", tag="pk")
                    for dc in range(n_dc):
                        nc.tensor.matmul(
                            pq[:],
                            lhsT=wq_sb[:, dc * 128:(dc + 1) * 128],
                            rhs=xt[dc][:, cs],
                            start=(dc == 0),
                            stop=(dc == n_dc - 1),
                        )
                    nc.vector.tensor_scalar_add(qt[:, cs], pq[:], bq_sb[:])

            # ---- attention ----
            # Per q-chunk there are 2*n_kc S^T slices (kc x head), exp'd in
            # PSUM chunks that ping-pong between a 4-slice [128, 2048] and a
            # 2-slice [128, 1024] tensor (strict A/B alternation globally so
            # each tag self-double-buffers).  E@V matmuls for a chunk are
            # emitted one chunk late so the PE always has S^T work queued
            # across chunk/q-boundaries.
            with (
                tc.tile_pool(name="ps_st", bufs=1, space="PSUM") as ps_st,
                tc.tile_pool(name="ps_o", bufs=1, space="PSUM") as ps_o,
                tc.tile_pool(name="etp", bufs=2) as etp,
                tc.tile_pool(name="outp", bufs=2) as outp,
            ):
                def emit_evs(evs):
                    for (ppo, ph, pkc, pet, poff) in evs:
                        nc.tensor.matmul(
                            ppo[:],
                            lhsT=v1[:, pkc * 256 + ph * 128:
                                    pkc * 256 + (ph + 1) * 128],
                            rhs=pet[:, poff:poff + 512],
                            start=(pkc == 0),
                            stop=(pkc == n_kc - 1),
                        )

                flip = [True]  # True -> next chunk uses tag A (4 slices)
                norm_prev = [None]

                for qc in range(n_qc):
                    qs = slice(qc * 512, (qc + 1) * 512)
                    po = [
                        ps_o.tile([128, 512], f32, name=f"po{h}", tag=f"po{h}")
                        for h in range(2)
                    ]
                    slices = [(kc, h) for kc in range(n_kc) for h in range(2)]
                    first_chunk = True
                    while slices:
                        nsl = min(4 if flip[0] else 2, len(slices))
                        tag = "stA" if flip[0] else "stB"
                        flip[0] = not flip[0]
                        w = nsl * 512
                        st_ps = ps_st.tile([128, w], f32, name="st_ps", tag=tag)
                        et = etp.tile([128, w], f16, name="et", tag="e" + tag)
                        batch, slices = slices[:nsl], slices[nsl:]
                        for si, (kc, h) in enumerate(batch):
                            hp = slice(h * 64, (h + 1) * 64)
                            nc.tensor.matmul(
                                st_ps[:, si * 512:(si + 1) * 512],
                                lhsT=kt[hp, kc * 128:(kc + 1) * 128],
                                rhs=qt[hp, qs],
                                start=True,
                                stop=True,
                            )
                        nc.scalar.activation(et[:], st_ps[:], AF.Exp, scale=0.125)
                        emit_evs([(po[h], h, kc, et, si * 512)
                                  for si, (kc, h) in enumerate(batch)])
                        if first_chunk and norm_prev[0] is not None:
                            norm_prev[0]()
                            norm_prev[0] = None
                        first_chunk = False

                    def make_norm(po=po, qc=qc):
                        def norm():
                            res = [
                                outp.tile([128, 128], f32, name=f"res{t}",
                                          tag=f"res{t}")
                                for t in range(4)
                            ]
                            for h in range(2):
                                ot = outp.tile([128, 512], f16, name="ot",
                                               tag="ot")
                                nc.vector.tensor_copy(ot[:], po[h][:])
                                for t in range(4):
                                    tp = outp.tile([128, 128], f16, name="tp",
                                                   tag="tp")
                                    nc.sync.dma_start_transpose(
                                        tp[:], ot[:, t * 128:(t + 1) * 128]
                                    )
                                    rcp = outp.tile([128, 1], f32, name="rcp",
                                                    tag="rcp")
                                    nc.vector.reciprocal(rcp[:], tp[:, 64:65])
                                    nc.vector.tensor_scalar_mul(
                                        res[t][:, h * 64:(h + 1) * 64],
                                        tp[:, 0:64], rcp[:],
                                    )
                            for t in range(4):
                                nc.sync.dma_start(
                                    out[qc * 512 + t * 128:
                                        qc * 512 + (t + 1) * 128, :],
                                    res[t][:],
                                )
                        return norm

                    norm_prev[0] = make_norm()
                norm_prev[0]()
    return nc


def _shard_inputs(x, Wq, bq, Wk, bk, Wv, bv):
    """Build the 8 per-core input maps from full inputs."""
    x = np.asarray(x, dtype=np.float32)
    in_maps = []
    for c in range(N_CORES):
        b, pair = c // 4, c % 4
        rows = slice(pair * 128, (pair + 1) * 128)
        wq_s = np.asarray(Wq)[rows, :].astype(np.float32)
        wk_s = np.asarray(Wk)[rows, :].astype(np.float32)
        wv_s = np.asarray(Wv)[rows, :].astype(np.float32)
        bq_s = np.asarray(bq)[rows].astype(np.float32)
        bk_s = np.asarray(bk)[rows].astype(np.float32)
        bv_s = np.asarray(bv)[rows].astype(np.float32)

        wvT = np.zeros((D_MODEL, 130), np.float32)
        wvT[:, 0:64] = wv_s[0:64].T
        wvT[:, 65:129] = wv_s[64:128].T
        bvb = np.zeros((128, 130), np.float32)
        bvb[:, 0:64] = bv_s[0:64]
        bvb[:, 64] = 1.0
        bvb[:, 65:129] = bv_s[64:128]
        bvb[:, 129] = 1.0

        in_maps.append({
            "xT": np.ascontiguousarray(x[c // 4].T).astype(np.float16),
            "wqT": np.ascontiguousarray(wq_s.T).astype(np.float16),
            "wkT": np.ascontiguousarray(wk_s.T).astype(np.float16),
            "wvT": wvT.astype(np.float16),
            "bq": bq_s.reshape(128, 1).copy(),
            "bk": bk_s.reshape(128, 1).copy(),
            "bvb": bvb,
        })
    return in_maps


def _gather(results):
    B, S, D = 2, S_FULL, D_MODEL
    out = np.empty((B, S, D), np.float32)
    for c in range(N_CORES):
        b, pair = c // 4, c % 4
        out[b, :, pair * 128:(pair + 1) * 128] = results[c]["out"]
    return out


def _install_profile_hook():
    """Provide antenv.axon_hooks (missing in this image) so that
    run_bass_kernel_spmd(trace=True) can capture NTFF profiles, using the
    same ctypes path trn_boot.py would have registered."""
    import sys, types, ctypes, contextlib

    if "antenv.axon_hooks" in sys.modules:
        return
    so_path = "/opt/axon/libaxon_pjrt.so"
    mod = types.ModuleType("antenv.axon_hooks")
    state = {"hook": None}
    mod.set_axon_ntff_profile_hook = lambda h: state.__setitem__("hook", h)
    mod.get_axon_ntff_profile_hook = lambda: state["hook"]
    sys.modules["antenv.axon_hooks"] = mod
    try:
        lib = ctypes.CDLL(so_path)
        if not hasattr(lib, "axon_start_nrt_profile"):
            return
        lib.axon_start_nrt_profile.argtypes = [
            ctypes.POINTER(ctypes.c_int64), ctypes.c_size_t]
        lib.axon_start_nrt_profile.restype = ctypes.c_int64
        lib.axon_stop_nrt_profile.argtypes = [ctypes.c_char_p]
        lib.axon_stop_nrt_profile.restype = ctypes.c_int64

        @contextlib.contextmanager
        def _hook(output_dir, device_ids):
            import jax
            jax.devices()
            if device_ids:
                ids = (ctypes.c_int64 * len(device_ids))(*device_ids)
                rc = lib.axon_start_nrt_profile(ids, len(device_ids))
            else:
                rc = lib.axon_start_nrt_profile(None, 0)
            if rc != 0:
                raise RuntimeError(f"axon_start_nrt_profile rc={rc}")
            try:
                yield
            finally:
                n = lib.axon_stop_nrt_profile(str(output_dir).encode())
                print(f"profile: {n} file(s) written to {output_dir}")

        state["hook"] = _hook
    except OSError:
        pass


def kernel(x, Wq, bq, Wk, bk, Wv, bv, trace=False):
    from concourse.bass_utils import run_bass_kernel_spmd

    if trace:
        _install_profile_hook()
    if "nc" not in _cached:
        nc = build_nc(S_FULL)
        nc.finalize()
        _cached["nc"] = nc
    nc = _cached["nc"]
    in_maps = _shard_inputs(x, Wq, bq, Wk, bk, Wv, bv)
    r = run_bass_kernel_spmd(nc, in_maps, list(range(N_CORES)), trace=trace)
    _cached["last_results"] = r
    return _gather(r.results)


# revision 17
# speedup vs baseline: 1.6745x; 1.6745x over previous
"""Multi-head attention (B=2, S=4096, D=512, H=8) on 8 NeuronCores.

Sharding: data-parallel on batch x head-pair-parallel.  Core c handles
batch b = c//4 and heads (2*(c%4), 2*(c%4)+1).  Each core computes its
[4096, 128] slice of the output; the host scatters inputs / gathers
outputs.

Per-core kernel (Bass/Tile), operands in fp16 (fp32 PSUM accumulate):
  - Pipelined prologue: x^T chunks DMA in while K^T and V projections
    run per 512-wide chunk; then Q^T projections.
  - Q^T/K^T are [128(d of 2 heads), S] with head dims on partitions; V
    sits in natural [k, d] layout padded to 128-wide FWL weight tiles
    with a ones column (so the E@V matmul also emits softmax row sums).
  - Attention per 512-wide q chunk / 128-wide k tile:
      S^T tile  = K^T.T @ Q^T  (two row-packed K=64 matmuls)
      E         = exp(S^T / 8) (ACT instrs over [128, 1536] PSUM chunks)
      O^T      += V1.T @ E     (PSUM accumulate, row 64 = row sums)
  - Output: PE transpose of O^T, reciprocal of row sums, per-partition
    scale, DMA out.
"""

import numpy as np

N_CORES = 8
S_FULL = 4096
D_MODEL = 512
HEAD = 64

_cached = {}


def build_nc(S=S_FULL):
    import concourse.bass as bass
    from concourse import bacc
    import concourse.mybir as mybir
    import concourse.tile as tile
    f32 = mybir.dt.float32
    f16 = mybir.dt.float16
    AF = mybir.ActivationFunctionType

    D = D_MODEL
    n_qc = S // 512     # 512-wide query chunks
    n_kc = S // 128     # 128-wide key tiles
    n_dc = D // 128     # 128-wide contraction chunks of D

    nc = bacc.Bacc()

    xT = nc.dram_tensor("xT", [D, S], f16, kind="ExternalInput")
    wqT = nc.dram_tensor("wqT", [D, 128], f16, kind="ExternalInput")
    wkT = nc.dram_tensor("wkT", [D, 128], f16, kind="ExternalInput")
    wvT = nc.dram_tensor("wvT", [D, 130], f16, kind="ExternalInput")
    bq = nc.dram_tensor("bq", [128, 1], f32, kind="ExternalInput")
    bk = nc.dram_tensor("bk", [128, 1], f32, kind="ExternalInput")
    bvb = nc.dram_tensor("bvb", [128, 130], f32, kind="ExternalInput")
    out = nc.dram_tensor("out", [S, 128], f32, kind="ExternalOutput")

    with tile.TileContext(nc) as tc:
        with (
            tc.tile_pool(name="consts", bufs=1) as consts,
            tc.tile_pool(name="persist", bufs=1) as persist,
        ):
            wq_sb = consts.tile([128, n_dc * 128], f16, name="wq_sb")
            wk_sb = consts.tile([128, n_dc * 128], f16, name="wk_sb")
            wv_sb = consts.tile([128, n_dc * 130], f16, name="wv_sb")
            bq_sb = consts.tile([128, 1], f32, name="bq_sb")
            bk_sb = consts.tile([128, 1], f32, name="bk_sb")
            bvb_sb = consts.tile([128, 130], f32, name="bvb_sb")
            for dc in range(n_dc):
                r = slice(dc * 128, (dc + 1) * 128)
                nc.sync.dma_start(wq_sb[:, dc * 128:(dc + 1) * 128], wqT[r, :])
                nc.sync.dma_start(wk_sb[:, dc * 128:(dc + 1) * 128], wkT[r, :])
                nc.sync.dma_start(wv_sb[:, dc * 130:(dc + 1) * 130], wvT[r, :])
            nc.sync.dma_start(bq_sb[:], bq[:, :])
            nc.sync.dma_start(bk_sb[:], bk[:, :])
            nc.sync.dma_start(bvb_sb[:], bvb[:, :])

            xt = [persist.tile([128, S], f16, name=f"xt{i}") for i in range(n_dc)]
            qt = persist.tile([128, S], f16, name="qt")
            kt = persist.tile([128, S], f16, name="kt")
            # V1[kc*256 + h*128 : +65] = [V_h | ones]; rest zero padding so
            # every E@V weight tile is a full 128-column (FWL-eligible) load.
            v1 = persist.tile([128, n_kc * 256], f16, name="v1")
            nc.vector.memset(v1[:], 0.0)

            # ---- prologue: x^T DMA + K/V projections pipelined per block --
            blk = 1024 if S >= 1024 else 512
            n_blk = S // blk
            with tc.tile_pool(name="pproj", bufs=2, space="PSUM") as pproj:
                for c in range(n_blk):
                    cs = slice(c * blk, (c + 1) * blk)
                    for dc in range(n_dc):
                        nc.sync.dma_start(
                            xt[dc][:, cs], xT[dc * 128:(dc + 1) * 128, cs]
                        )
                    for half in range(blk // 512):
                        hs = slice(c * blk + half * 512, c * blk + (half + 1) * 512)
                        pk = pproj.tile([128, 512], f32, name="pk", tag="pk")
                        for dc in range(n_dc):
                            nc.tensor.matmul(
                                pk[:],
                                lhsT=wk_sb[:, dc * 128:(dc + 1) * 128],
                                rhs=xt[dc][:, hs],
                                start=(dc == 0),
                                stop=(dc == n_dc - 1),
                            )
                        nc.vector.tensor_scalar_add(kt[:, hs], pk[:], bk_sb[:])
                    for sti in range(blk // 128):
                        st_ = c * (blk // 128) + sti
                        ss = slice(st_ * 128, (st_ + 1) * 128)
                        pv = pproj.tile([128, 130], f32, name="pv", tag="pv")
                        for dc in range(n_dc):
                            nc.tensor.matmul(
                                pv[:],
                                lhsT=xt[dc][:, ss],
                                rhs=wv_sb[:, dc * 130:(dc + 1) * 130],
                                start=(dc == 0),
                                stop=(dc == n_dc - 1),
                            )
                        for h in range(2):
                            nc.vector.tensor_add(
                                v1[:, st_ * 256 + h * 128: st_ * 256 + h * 128 + 65],
                                pv[:, h * 65:(h + 1) * 65],
                                bvb_sb[:, h * 65:(h + 1) * 65],
                            )
                for c in range(n_qc):
                    cs = slice(c * 512, (c + 1) * 512)
                    pq = pproj.tile([128, 512], f32, name="pq", tag="pk")
                    for dc in range(n_dc):
                        nc.tensor.matmul(
                            pq[:],
                            lhsT=wq_sb[:, dc * 128:(dc + 1) * 128],
                            rhs=xt[dc][:, cs],
                            start=(dc == 0),
                            stop=(dc == n_dc - 1),
                        )
                    nc.vector.tensor_scalar_add(qt[:, cs], pq[:], bq_sb[:])

            # ---- attention ----
            # Per q-chunk there are 2*n_kc S^T slices (kc x head), exp'd in
            # PSUM chunks that ping-pong between a 4-slice [128, 2048] and a
            # 2-slice [128, 1024] tensor (strict A/B alternation globally so
            # each tag self-double-buffers).  E@V matmuls for a chunk are
            # emitted one chunk late so the PE always has S^T work queued
            # across chunk/q-boundaries.
            with (
                tc.tile_pool(name="ps_st", bufs=2, space="PSUM") as ps_st,
                tc.tile_pool(name="ps_o", bufs=1, space="PSUM") as ps_o,
                tc.tile_pool(name="etp", bufs=3) as etp,
                tc.tile_pool(name="outp", bufs=2) as outp,
            ):
                def emit_evs(evs):
                    for (ppo, ph, pkc, pet, poff) in evs:
                        nc.tensor.matmul(
                            ppo[:],
                            lhsT=v1[:, pkc * 256 + ph * 128:
                                    pkc * 256 + (ph + 1) * 128],
                            rhs=pet[:, poff:poff + 512],
                            start=(pkc == 0),
                            stop=(pkc == n_kc - 1),
                        )

                for qc in range(n_qc):
                    qs = slice(qc * 512, (qc + 1) * 512)
                    po = [
                        ps_o.tile([128, 512], f32, name=f"po{h}", tag=f"po{h}")
                        for h in range(2)
                    ]
                    slices = [(kc, h) for kc in range(n_kc) for h in range(2)]
                    while slices:
                        nsl = min(3, len(slices))
                        w = nsl * 512
                        st_ps = ps_st.tile([128, w], f32, name="st_ps", tag="st")
                        et = etp.tile([128, w], f16, name="et", tag="et")
                        batch, slices = slices[:nsl], slices[nsl:]
                        for si, (kc, h) in enumerate(batch):
                            hp = slice(h * 64, (h + 1) * 64)
                            nc.tensor.matmul(
                                st_ps[:, si * 512:(si + 1) * 512],
                                lhsT=kt[hp, kc * 128:(kc + 1) * 128],
                                rhs=qt[hp, qs],
                                start=True,
                                stop=True,
                            )
                        nc.scalar.activation(et[:], st_ps[:], AF.Exp, scale=0.125)
                        emit_evs([(po[h], h, kc, et, si * 512)
                                  for si, (kc, h) in enumerate(batch)])
                    # normalize + transpose + store
                    res = [
                        outp.tile([128, 128], f32, name=f"res{t}", tag=f"res{t}")
                        for t in range(4)
                    ]
                    for h in range(2):
                        ot = outp.tile([128, 512], f16, name="ot", tag="ot")
                        nc.vector.tensor_copy(ot[:], po[h][:])
                        for t in range(4):
                            tp = outp.tile([128, 128], f16, name="tp", tag="tp")
                            nc.sync.dma_start_transpose(
                                tp[:], ot[:, t * 128:(t + 1) * 128]
                            )
                            rcp = outp.tile([128, 1], f32, name="rcp", tag="rcp")
                            nc.vector.reciprocal(rcp[:], tp[:, 64:65])
                            nc.vector.tensor_scalar_mul(
                                res[t][:, h * 64:(h + 1) * 64], tp[:, 0:64], rcp[:]
                            )
                    for t in range(4):
                        nc.sync.dma_start(
                            out[qc * 512 + t * 128: qc * 512 + (t + 1) * 128, :],
                            res[t][:],
                        )
    return nc


def _shard_inputs(x, Wq, bq, Wk, bk, Wv, bv):
    """Build the 8 per-core input maps from full inputs."""
    x = np.asarray(x, dtype=np.float32)
    in_maps = []
    for c in range(N_CORES):
        b, pair = c // 4, c % 4
        rows = slice(pair * 128, (pair + 1) * 128)
        wq_s = np.asarray(Wq)[rows, :].astype(np.float32)
        wk_s = np.asarray(Wk)[rows, :].astype(np.float32)
        wv_s = np.asarray(Wv)[rows, :].astype(np.float32)
        bq_s = np.asarray(bq)[rows].astype(np.float32)
        bk_s = np.asarray(bk)[rows].astype(np.float32)
        bv_s = np.asarray(bv)[rows].astype(np.float32)

        wvT = np.zeros((D_MODEL, 130), np.float32)
        wvT[:, 0:64] = wv_s[0:64].T
        wvT[:, 65:129] = wv_s[64:128].T
        bvb = np.zeros((128, 130), np.float32)
        bvb[:, 0:64] = bv_s[0:64]
        bvb[:, 64] = 1.0
        bvb[:, 65:129] = bv_s[64:128]
        bvb[:, 129] = 1.0

        in_maps.append({
            "xT": np.ascontiguousarray(x[c // 4].T).astype(np.float16),
            "wqT": np.ascontiguousarray(wq_s.T).astype(np.float16),
            "wkT": np.ascontiguousarray(wk_s.T).astype(np.float16),
            "wvT": wvT.astype(np.float16),
            "bq": bq_s.reshape(128, 1).copy(),
            "bk": bk_s.reshape(128, 1).copy(),
            "bvb": bvb,
        })
    return in_maps


def _gather(results):
    B, S, D = 2, S_FULL, D_MODEL
    out = np.empty((B, S, D), np.float32)
    for c in range(N_CORES):
        b, pair = c // 4, c % 4
        out[b, :, pair * 128:(pair + 1) * 128] = results[c]["out"]
    return out


def _install_profile_hook():
    """Provide antenv.axon_hooks (missing in this image) so that
    run_bass_kernel_spmd(trace=True) can capture NTFF profiles, using the
    same ctypes path trn_boot.py would have registered."""
    import sys, types, ctypes, contextlib

    if "antenv.axon_hooks" in sys.modules:
        return
    so_path = "/opt/axon/libaxon_pjrt.so"
    mod = types.ModuleType("antenv.axon_hooks")
    state = {"hook": None}
    mod.set_axon_ntff_profile_hook = lambda h: state.__setitem__("hook", h)
    mod.get_axon_ntff_profile_hook = lambda: state["hook"]
    sys.modules["antenv.axon_hooks"] = mod
    try:
        lib = ctypes.CDLL(so_path)
        if not hasattr(lib, "axon_start_nrt_profile"):
            return
        lib.axon_start_nrt_profile.argtypes = [
            ctypes.POINTER(ctypes.c_int64), ctypes.c_size_t]
        lib.axon_start_nrt_profile.restype = ctypes.c_int64
        lib.axon_stop_nrt_profile.argtypes = [ctypes.c_char_p]
        lib.axon_stop_nrt_profile.restype = ctypes.c_int64

        @contextlib.contextmanager
        def _hook(output_dir, device_ids):
            import jax
            jax.devices()
            if device_ids:
                ids = (ctypes.c_int64 * len(device_ids))(*device_ids)
                rc = lib.axon_start_nrt_profile(ids, len(device_ids))
            else:
                rc = lib.axon_start_nrt_profile(None, 0)
            if rc != 0:
                raise RuntimeError(f"axon_start_nrt_profile rc={rc}")
            try:
                yield
            finally:
                n = lib.axon_stop_nrt_profile(str(output_dir).encode())
                print(f"profile: {n} file(s) written to {output_dir}")

        state["hook"] = _hook
    except OSError:
        pass


def kernel(x, Wq, bq, Wk, bk, Wv, bv, trace=False):
    from concourse.bass_utils import run_bass_kernel_spmd

    if trace:
        _install_profile_hook()
    if "nc" not in _cached:
        nc = build_nc(S_FULL)
        nc.finalize()
        _cached["nc"] = nc
    nc = _cached["nc"]
    in_maps = _shard_inputs(x, Wq, bq, Wk, bk, Wv, bv)
    r = run_bass_kernel_spmd(nc, in_maps, list(range(N_CORES)), trace=trace)
    _cached["last_results"] = r
    return _gather(r.results)


# revision 19
# speedup vs baseline: 1.7240x; 1.0295x over previous
"""Multi-head attention (B=2, S=4096, D=512, H=8) on 8 NeuronCores.

Sharding: data-parallel on batch x head-pair-parallel.  Core c handles
batch b = c//4 and heads (2*(c%4), 2*(c%4)+1).  Each core computes its
[4096, 128] slice of the output; the host scatters inputs / gathers
outputs.

Per-core kernel (Bass/Tile), operands in fp16 (fp32 PSUM accumulate):
  - Pipelined prologue: x^T chunks DMA in while K^T and V projections
    run per 512-wide chunk; then Q^T projections.
  - Q^T/K^T are [128(d of 2 heads), S] with head dims on partitions; V
    sits in natural [k, d] layout padded to 128-wide FWL weight tiles
    with a ones column (so the E@V matmul also emits softmax row sums).
  - Attention per 512-wide q chunk / 128-wide k tile:
      S^T tile  = K^T.T @ Q^T  (two row-packed K=64 matmuls)
      E         = exp(S^T / 8) (ACT instrs over [128, 1536] PSUM chunks)
      O^T      += V1.T @ E     (PSUM accumulate, row 64 = row sums)
  - Output: PE transpose of O^T, reciprocal of row sums, per-partition
    scale, DMA out.
"""

import numpy as np

N_CORES = 8
S_FULL = 4096
D_MODEL = 512
HEAD = 64

_cached = {}


def build_nc(S=S_FULL):
    import concourse.bass as bass
    from concourse import bacc
    import concourse.mybir as mybir
    import concourse.tile as tile
    from concourse.masks import make_identity
    f32 = mybir.dt.float32
    f16 = mybir.dt.float16
    AF = mybir.ActivationFunctionType

    D = D_MODEL
    n_qc = S // 512     # 512-wide query chunks
    n_kc = S // 128     # 128-wide key tiles
    n_dc = D // 128     # 128-wide contraction chunks of D

    nc = bacc.Bacc()

    xT = nc.dram_tensor("xT", [D, S], f16, kind="ExternalInput")
    wqT = nc.dram_tensor("wqT", [D, 128], f16, kind="ExternalInput")
    wkT = nc.dram_tensor("wkT", [D, 128], f16, kind="ExternalInput")
    wvT = nc.dram_tensor("wvT", [D, 130], f16, kind="ExternalInput")
    bq = nc.dram_tensor("bq", [128, 1], f32, kind="ExternalInput")
    bk = nc.dram_tensor("bk", [128, 1], f32, kind="ExternalInput")
    bvb = nc.dram_tensor("bvb", [128, 130], f32, kind="ExternalInput")
    out = nc.dram_tensor("out", [S, 128], f32, kind="ExternalOutput")

    with tile.TileContext(nc) as tc:
        with (
            tc.tile_pool(name="consts", bufs=1) as consts,
            tc.tile_pool(name="persist", bufs=1) as persist,
        ):
            ident = consts.tile([128, 128], f16, name="ident")
            make_identity(nc, ident)
            wq_sb = consts.tile([128, n_dc * 128], f16, name="wq_sb")
            wk_sb = consts.tile([128, n_dc * 128], f16, name="wk_sb")
            wv_sb = consts.tile([128, n_dc * 130], f16, name="wv_sb")
            bq_sb = consts.tile([128, 1], f32, name="bq_sb")
            bk_sb = consts.tile([128, 1], f32, name="bk_sb")
            bvb_sb = consts.tile([128, 130], f32, name="bvb_sb")
            for dc in range(n_dc):
                r = slice(dc * 128, (dc + 1) * 128)
                nc.sync.dma_start(wq_sb[:, dc * 128:(dc + 1) * 128], wqT[r, :])
                nc.sync.dma_start(wk_sb[:, dc * 128:(dc + 1) * 128], wkT[r, :])
                nc.sync.dma_start(wv_sb[:, dc * 130:(dc + 1) * 130], wvT[r, :])
            nc.sync.dma_start(bq_sb[:], bq[:, :])
            nc.sync.dma_start(bk_sb[:], bk[:, :])
            nc.sync.dma_start(bvb_sb[:], bvb[:, :])

            xt = [persist.tile([128, S], f16, name=f"xt{i}") for i in range(n_dc)]
            qt = persist.tile([128, S], f16, name="qt")
            kt = persist.tile([128, S], f16, name="kt")
            # V1[kc*256 + h*128 : +65] = [V_h | ones]; rest zero padding so
            # every E@V weight tile is a full 128-column (FWL-eligible) load.
            v1 = persist.tile([128, n_kc * 256], f16, name="v1")
            nc.vector.memset(v1[:], 0.0)

            # ---- prologue: x^T DMA + K/V projections pipelined per block --
            if S >= 2048:
                blocks = [(0, 512), (512, 512), (1024, 1024)]
                o = 2048
                while o < S:
                    blocks.append((o, 1024))
                    o += 1024
            else:
                blocks = [(o, 512) for o in range(0, S, 512)]
            with tc.tile_pool(name="pproj", bufs=2, space="PSUM") as pproj:
                for (boff, blk) in blocks:
                    cs = slice(boff, boff + blk)
                    for dc in range(n_dc):
                        nc.sync.dma_start(
                            xt[dc][:, cs], xT[dc * 128:(dc + 1) * 128, cs]
                        )
                    for half in range(blk // 512):
                        hs = slice(boff + half * 512, boff + (half + 1) * 512)
                        pk = pproj.tile([128, 512], f32, name="pk", tag="pk")
                        for dc in range(n_dc):
                            nc.tensor.matmul(
                                pk[:],
                                lhsT=wk_sb[:, dc * 128:(dc + 1) * 128],
                                rhs=xt[dc][:, hs],
                                start=(dc == 0),
                                stop=(dc == n_dc - 1),
                            )
                        nc.vector.tensor_scalar_add(kt[:, hs], pk[:], bk_sb[:])
                    for sti in range(blk // 128):
                        st_ = boff // 128 + sti
                        ss = slice(st_ * 128, (st_ + 1) * 128)
                        pv = pproj.tile([128, 130], f32, name="pv", tag="pv")
                        for dc in range(n_dc):
                            nc.tensor.matmul(
                                pv[:],
                                lhsT=xt[dc][:, ss],
                                rhs=wv_sb[:, dc * 130:(dc + 1) * 130],
                                start=(dc == 0),
                                stop=(dc == n_dc - 1),
                            )
                        for h in range(2):
                            nc.vector.tensor_add(
                                v1[:, st_ * 256 + h * 128: st_ * 256 + h * 128 + 65],
                                pv[:, h * 65:(h + 1) * 65],
                                bvb_sb[:, h * 65:(h + 1) * 65],
                            )
                for c in range(n_qc):
                    cs = slice(c * 512, (c + 1) * 512)
                    pq = pproj.tile([128, 512], f32, name="pq", tag="pk")
                    for dc in range(n_dc):
                        nc.tensor.matmul(
                            pq[:],
                            lhsT=wq_sb[:, dc * 128:(dc + 1) * 128],
                            rhs=xt[dc][:, cs],
                            start=(dc == 0),
                            stop=(dc == n_dc - 1),
                        )
                    nc.vector.tensor_scalar_add(qt[:, cs], pq[:], bq_sb[:])

            # ---- attention ----
            # Per q-chunk there are 2*n_kc S^T slices (kc x head), exp'd in
            # PSUM chunks that ping-pong between a 4-slice [128, 2048] and a
            # 2-slice [128, 1024] tensor (strict A/B alternation globally so
            # each tag self-double-buffers).  E@V matmuls for a chunk are
            # emitted one chunk late so the PE always has S^T work queued
            # across chunk/q-boundaries.
            with (
                tc.tile_pool(name="ps_st", bufs=2, space="PSUM") as ps_st,
                tc.tile_pool(name="ps_o", bufs=1, space="PSUM") as ps_o,
                tc.tile_pool(name="etp", bufs=3) as etp,
                tc.tile_pool(name="outp", bufs=2) as outp,
            ):
                def emit_evs(evs):
                    for (ppo, ph, pkc, pet, poff) in evs:
                        nc.tensor.matmul(
                            ppo[:],
                            lhsT=v1[:, pkc * 256 + ph * 128:
                                    pkc * 256 + (ph + 1) * 128],
                            rhs=pet[:, poff:poff + 512],
                            start=(pkc == 0),
                            stop=(pkc == n_kc - 1),
                        )

                for qc in range(n_qc):
                    qs = slice(qc * 512, (qc + 1) * 512)
                    po = [
                        ps_o.tile([128, 512], f32, name=f"po{h}", tag=f"po{h}")
                        for h in range(2)
                    ]
                    slices = [(kc, h) for kc in range(n_kc) for h in range(2)]
                    while slices:
                        nsl = min(3, len(slices))
                        w = nsl * 512
                        st_ps = ps_st.tile([128, w], f32, name="st_ps", tag="st")
                        et = etp.tile([128, w], f16, name="et", tag="et")
                        batch, slices = slices[:nsl], slices[nsl:]
                        for si, (kc, h) in enumerate(batch):
                            hp = slice(h * 64, (h + 1) * 64)
                            nc.tensor.matmul(
                                st_ps[:, si * 512:(si + 1) * 512],
                                lhsT=kt[hp, kc * 128:(kc + 1) * 128],
                                rhs=qt[hp, qs],
                                start=True,
                                stop=True,
                            )
                        nc.scalar.activation(et[:], st_ps[:], AF.Exp, scale=0.125)
                        emit_evs([(po[h], h, kc, et, si * 512)
                                  for si, (kc, h) in enumerate(batch)])
                    # normalize + transpose + store
                    res = [
                        outp.tile([128, 128], f32, name=f"res{t}", tag=f"res{t}")
                        for t in range(4)
                    ]
                    last = qc == n_qc - 1
                    for h in range(2):
                        ot = outp.tile([128, 512], f16, name="ot", tag="ot")
                        nc.vector.tensor_copy(ot[:], po[h][:])
                        for t in range(4):
                            if last:
                                # PE transpose (xbar DMA transposes would
                                # serialize on the exposed kernel tail)
                                pt = ps_st.tile([128, 65], f16, name="pt",
                                                tag="st")
                                nc.tensor.transpose(
                                    pt[:],
                                    ot[0:65, t * 128:(t + 1) * 128],
                                    ident[0:65, 0:65],
                                )
                                src = pt
                            else:
                                tp = outp.tile([128, 128], f16, name="tp",
                                               tag="tp")
                                nc.sync.dma_start_transpose(
                                    tp[:], ot[:, t * 128:(t + 1) * 128]
                                )
                                src = tp
                            rcp = outp.tile([128, 1], f32, name="rcp", tag="rcp")
                            nc.vector.reciprocal(rcp[:], src[:, 64:65])
                            nc.vector.tensor_scalar_mul(
                                res[t][:, h * 64:(h + 1) * 64], src[:, 0:64], rcp[:]
                            )
                    for t in range(4):
                        nc.sync.dma_start(
                            out[qc * 512 + t * 128: qc * 512 + (t + 1) * 128, :],
                            res[t][:],
                        )
    return nc


def _shard_inputs(x, Wq, bq, Wk, bk, Wv, bv):
    """Build the 8 per-core input maps from full inputs."""
    x = np.asarray(x, dtype=np.float32)
    in_maps = []
    for c in range(N_CORES):
        b, pair = c // 4, c % 4
        rows = slice(pair * 128, (pair + 1) * 128)
        wq_s = np.asarray(Wq)[rows, :].astype(np.float32)
        wk_s = np.asarray(Wk)[rows, :].astype(np.float32)
        wv_s = np.asarray(Wv)[rows, :].astype(np.float32)
        bq_s = np.asarray(bq)[rows].astype(np.float32)
        bk_s = np.asarray(bk)[rows].astype(np.float32)
        bv_s = np.asarray(bv)[rows].astype(np.float32)

        wvT = np.zeros((D_MODEL, 130), np.float32)
        wvT[:, 0:64] = wv_s[0:64].T
        wvT[:, 65:129] = wv_s[64:128].T
        bvb = np.zeros((128, 130), np.float32)
        bvb[:, 0:64] = bv_s[0:64]
        bvb[:, 64] = 1.0
        bvb[:, 65:129] = bv_s[64:128]
        bvb[:, 129] = 1.0

        in_maps.append({
            "xT": np.ascontiguousarray(x[c // 4].T).astype(np.float16),
            "wqT": np.ascontiguousarray(wq_s.T).astype(np.float16),
            "wkT": np.ascontiguousarray(wk_s.T).astype(np.float16),
            "wvT": wvT.astype(np.float16),
            "bq": bq_s.reshape(128, 1).copy(),
            "bk": bk_s.reshape(128, 1).copy(),
            "bvb": bvb,
        })
    return in_maps


def _gather(results):
    B, S, D = 2, S_FULL, D_MODEL
    out = np.empty((B, S, D), np.float32)
    for c in range(N_CORES):
        b, pair = c // 4, c % 4
        out[b, :, pair * 128:(pair + 1) * 128] = results[c]["out"]
    return out


def _install_profile_hook():
    """Provide antenv.axon_hooks (missing in this image) so that
    run_bass_kernel_spmd(trace=True) can capture NTFF profiles, using the
    same ctypes path trn_boot.py would have registered."""
    import sys, types, ctypes, contextlib

    if "antenv.axon_hooks" in sys.modules:
        return
    so_path = "/opt/axon/libaxon_pjrt.so"
    mod = types.ModuleType("antenv.axon_hooks")
    state = {"hook": None}
    mod.set_axon_ntff_profile_hook = lambda h: state.__setitem__("hook", h)
    mod.get_axon_ntff_profile_hook = lambda: state["hook"]
    sys.modules["antenv.axon_hooks"] = mod
    try:
        lib = ctypes.CDLL(so_path)
        if not hasattr(lib, "axon_start_nrt_profile"):
            return
        lib.axon_start_nrt_profile.argtypes = [
            ctypes.POINTER(ctypes.c_int64), ctypes.c_size_t]
        lib.axon_start_nrt_profile.restype = ctypes.c_int64
        lib.axon_stop_nrt_profile.argtypes = [ctypes.c_char_p]
        lib.axon_stop_nrt_profile.restype = ctypes.c_int64

        @contextlib.contextmanager
        def _hook(output_dir, device_ids):
            import jax
            jax.devices()
            if device_ids:
                ids = (ctypes.c_int64 * len(device_ids))(*device_ids)
                rc = lib.axon_start_nrt_profile(ids, len(device_ids))
            else:
                rc = lib.axon_start_nrt_profile(None, 0)
            if rc != 0:
                raise RuntimeError(f"axon_start_nrt_profile rc={rc}")
            try:
                yield
            finally:
                n = lib.axon_stop_nrt_profile(str(output_dir).encode())
                print(f"profile: {n} file(s) written to {output_dir}")

        state["hook"] = _hook
    except OSError:
        pass


def kernel(x, Wq, bq, Wk, bk, Wv, bv, trace=False):
    from concourse.bass_utils import run_bass_kernel_spmd

    if trace:
        _install_profile_hook()
    if "nc" not in _cached:
        nc = build_nc(S_FULL)
        nc.finalize()
        _cached["nc"] = nc
    nc = _cached["nc"]
    in_maps = _shard_inputs(x, Wq, bq, Wk, bk, Wv, bv)
    r = run_bass_kernel_spmd(nc, in_maps, list(range(N_CORES)), trace=trace)
    _cached["last_results"] = r
    return _gather(r.results)
